# revision 7
# baseline (speedup 1.0000x reference)
"""Trainium2 Bass kernel for nn_BilinearAttention (GNN message passing).

Self-contained: takes FULL inputs, shards across 8 NeuronCores internally,
returns the FULL [50000, 512] float32 output.

Strategy (per core, 1/8 node+edge shard):
- PE-transpose x tiles; matmuls produce a combined bf16 [q_l|k_l] table shard,
  ego/global score rows (transposed), and the x_bar partial.
- One AllGather shares the combined table (zero row appended per rank so a
  two-pass int16 dma_gather with clamped indices can cover all 50000 rows);
  one AllReduce combines x_bar.
- GPSIMD dma_gather pulls per-edge q/k rows (lo/hi passes, invalid indices
  clamp to zero rows), DVE combines, multiplies, and segment-sums per node.
- PE matmuls apply the three value projections and the shared normalizer;
  DVE scales and adds the bias; results DMA to the output shard.
"""
import sys
sys.path.insert(0, "/opt/trn_rl_repo")
import numpy as np

import concourse.ap_utils as ap_utils
import concourse.bacc as bacc
import concourse.tile as tile
from concourse import bass, mybir
from concourse.bass import round_up_to_multiple
from concourse.bass_utils import run_bass_kernel_spmd
from concourse.masks import make_identity

F32 = mybir.dt.float32
BF16 = mybir.dt.bfloat16
I16 = mybir.dt.int16
AF = mybir.ActivationFunctionType
ALU = mybir.AluOpType


# ----------------------------------------------------------------------------
# low-level: dma_gather emitter (allows payload < row stride)
# ----------------------------------------------------------------------------
def _dma_gather_hbm(eng, out_ap, in_ap, idxs_ap, num_idxs, num_idxs_reg,
                    elem_size, elem_step, queue_num=0, single_packet=False):
    eng._assert_queue_num(queue_num)
    assert idxs_ap.dtype == mybir.dt.int16
    assert in_ap.dtype == out_ap.dtype
    assert ap_utils.ap_is_contiguous(out_ap.ap[1:])
    assert ap_utils.ap_is_contiguous(idxs_ap.ap[1:])
    assert in_ap.ap[-1][1] == out_ap.ap[-1][1] == elem_size
    assert out_ap.ap[0][1] * out_ap.ap[1][1] == round_up_to_multiple(num_idxs, 128)
    assert in_ap.ap[0][0] == elem_step
    stride_bytes = elem_step * mybir.dt.size(in_ap.dtype)
    assert stride_bytes % 256 == 0
    stride_bytes_256 = stride_bytes // 256
    assert 0 < stride_bytes_256 < 256
    _in_ap = eng.lower_ap_dma(in_ap, for_custom_bir_dma=True)
    _idxs_ap = eng.lower_ap(idxs_ap)
    _out_ap = eng.lower_ap(out_ap)
    return eng.add_instruction(
        mybir.InstDMAGatherAnt(
            name=eng.bass.get_next_instruction_name(),
            ins=[*_in_ap, _idxs_ap, eng.lower_val_access(eng.to_reg(num_idxs_reg))],
            outs=[_out_ap],
            transpose=False,
            num_idxs=num_idxs,
            elem_size=elem_size,
            stride_bytes_256=stride_bytes_256,
            gen_mode=0,
            single_packet=single_packet,
            queue_num=queue_num,
            sbuf_tokens_per_rank=0,
            sbuf_free_dim_per_rank=0,
            sbuf_free_dim_pad_per_rank=0,
            sbuf_byte_offset=0,
        )
    )


# ----------------------------------------------------------------------------
# configuration
# ----------------------------------------------------------------------------
class Cfg:
    def __init__(self, N=50000, DIN=512, DEG=32, DL=64, DE=32, DG=32, DOUT=512,
                 CORES=8, J=8192):
        self.N, self.DIN, self.DEG = N, DIN, DEG
        self.DL, self.DE, self.DG, self.DOUT = DL, DE, DG, DOUT
        self.CORES = CORES
        self.NS = N // CORES
        self.ES = self.NS * DEG
        self.NB = self.NS + 1
        self.NTOT = self.NB * CORES
        zrows = [r * self.NB + self.NS for r in range(CORES)]
        self.SPLIT = max(z for z in zrows if z <= 32767)
        assert self.NTOT - self.SPLIT - 1 <= 32767
        self.J = J
        self.C = J // 128
        self.NPP = self.C // DEG
        assert self.C % DEG == 0 and self.NPP in (1, 2)
        self.NT_G = 128 * self.NPP
        self.G_TILES = (self.NS + self.NT_G - 1) // self.NT_G
        self.N_TILES = (self.NS + 127) // 128
        self.INV = 1.0 / (DIN * DIN)


# ----------------------------------------------------------------------------
# host-side sharding / index layout
# ----------------------------------------------------------------------------
def prep_core_inputs(cfg, adj, x, c):
    NS, ES, DEG, J, C = cfg.NS, cfg.ES, cfg.DEG, cfg.J, cfg.C
    t_idx = np.asarray(adj[1, c * ES:(c + 1) * ES], dtype=np.int64)
    s_idx = np.asarray(adj[0, c * ES:(c + 1) * ES], dtype=np.int64)
    t_ph = t_idx + t_idx // NS
    s_ph = s_idx + s_idx // NS

    def tiles_for(vals, pad):
        out = np.empty((cfg.G_TILES, 128, J // 16), dtype=np.int16)
        p = np.arange(128)[:, None]
        cc = np.arange(C)[None, :]
        for g in range(cfg.G_TILES):
            node = g * cfg.NT_G + 128 * (cc // DEG) + p
            edge = node * DEG + (cc % DEG)
            valid = node < NS
            v = np.where(valid, vals[np.where(valid, edge, 0)], pad).astype(np.int16)
            w = v.T.flatten()
            out[g] = np.tile(w.reshape(J // 16, 16).T, (8, 1))
        return out

    lo = lambda ph: np.minimum(ph, cfg.SPLIT)
    hi = lambda ph: np.maximum(ph - cfg.SPLIT, 0)
    return {
        "x_shard": np.ascontiguousarray(x[c * NS:(c + 1) * NS], dtype=np.float32),
        "tlo": tiles_for(lo(t_ph), cfg.SPLIT),
        "thi": tiles_for(hi(t_ph), 0),
        "slo": tiles_for(lo(s_ph), cfg.SPLIT),
        "shi": tiles_for(hi(s_ph), 0),
    }


# ----------------------------------------------------------------------------
# device program
# ----------------------------------------------------------------------------
def build(cfg, fake_cc=False):
    NS, DIN, DL, DE, DG, DOUT = cfg.NS, cfg.DIN, cfg.DL, cfg.DE, cfg.DG, cfg.DOUT
    J, C, NPP, DEG = cfg.J, cfg.C, cfg.NPP, cfg.DEG
    KC = DIN // 128
    DQK = DL + DL
    DSC = DE + DG

    nc = bacc.Bacc("TRN2", target_bir_lowering=False, debug=False,
                   num_devices=1 if fake_cc else cfg.CORES)

    t_x = nc.dram_tensor("x_shard", [NS, DIN], F32, kind="ExternalInput").ap()
    t_idx = {nm: nc.dram_tensor(nm, [cfg.G_TILES, 128, J // 16], I16,
                                kind="ExternalInput").ap()
             for nm in ("tlo", "thi", "slo", "shi")}
    wts = {}
    for nm, shp in (("w_ego", [DE, DIN]), ("v_ego_w", [DOUT, DE]),
                    ("q_local_w", [DL, DIN]), ("k_local_w", [DL, DIN]),
                    ("v_local_w", [DOUT, DL]), ("q_global_w", [DG, DIN]),
                    ("k_global_w", [DG, DIN]), ("v_global_w", [DOUT, DG]),
                    ("bias_b", [1, DOUT])):
        wts[nm] = nc.dram_tensor(nm, shp, F32, kind="ExternalInput").ap()
    t_res = nc.dram_tensor("res", [NS, DOUT], F32, kind="ExternalOutput").ap()

    rg = [list(range(cfg.CORES))]

    with tile.TileContext(nc) as tc:
        with (
            tc.tile_pool(name="dram", bufs=1, space="DRAM") as dram,
            tc.tile_pool(name="persist", bufs=1) as ps,
            tc.tile_pool(name="wtmp", bufs=2) as wtmp,
            tc.tile_pool(name="psA", bufs=2, space="PSUM") as psA,
            tc.tile_pool(name="psB", bufs=2, space="PSUM") as psB,
            tc.tile_pool(name="psX", bufs=1, space="PSUM") as psX,
            tc.tile_pool(name="psS", bufs=1, space="PSUM") as psS,
            tc.tile_pool(name="psO", bufs=2, space="PSUM") as psO,
            tc.tile_pool(name="s1", bufs=2) as s1p,
            tc.tile_pool(name="gat", bufs=2) as gp,
            tc.tile_pool(name="fin", bufs=2) as fp,
        ):
            cc_in = dram.tile([cfg.NB, DQK], BF16)
            cc_out = dram.tile([cfg.NTOT, DQK], BF16)
            ar_in = dram.tile([1, DIN], F32)
            ar_out = dram.tile([1, DIN], F32)

            # ---- constants & weights ----
            ident = ps.tile([128, 128], F32)
            make_identity(nc, ident[:])
            ones_col = ps.tile([128, 1], F32)
            nc.vector.memset(ones_col[:], 1.0)
            ones_row = ps.tile([1, 128], F32)
            nc.vector.memset(ones_row[:], 1.0)
            zrow_bf = ps.tile([1, DQK], BF16)
            nc.vector.memset(zrow_bf[:], 0.0)
            nc.sync.dma_start(cc_in[NS:NS + 1, :], zrow_bf[:])

            def load_w(nm):
                t = wtmp.tile(list(wts[nm].shape), F32, tag="wld")
                nc.sync.dma_start(t[:], wts[nm])
                return t

            def nonneg(dst_ap, src_ap, P, F):
                tmin = wtmp.tile([P, F], F32, tag="nn_min")
                tmax = wtmp.tile([P, F], F32, tag="nn_max")
                nc.vector.tensor_scalar_min(tmin[:P, :F], src_ap, 0.0)
                nc.vector.tensor_scalar_max(tmax[:P, :F], src_ap, 0.0)
                nc.scalar.activation(tmin[:P, :F], tmin[:P, :F], AF.Exp)
                nc.vector.tensor_add(dst_ap, tmin[:P, :F], tmax[:P, :F])

            def normed(dst_ap, src_ap, P, F, extra_scale):
                sg = wtmp.tile([P, F], F32, tag="nrm_sig")
                rs = wtmp.tile([P, 1], F32, tag="nrm_rs")
                nc.scalar.activation(sg[:P, :F], src_ap, AF.Sigmoid)
                nc.vector.tensor_reduce(rs[:P, :1], sg[:P, :F], mybir.AxisListType.X, ALU.add)
                pt = psA.tile([1, 1], F32, tag="a")
                nc.tensor.matmul(pt[:1, :1], rs[:P, :1], ones_col[:P, :1], start=True, stop=True)
                tot = wtmp.tile([1, 1], F32, tag="nrm_tot")
                nc.vector.reciprocal(tot[:1, :1], pt[:1, :1])
                pb = psA.tile([P, 1], F32, tag="a")
                nc.tensor.matmul(pb[:P, :1], ones_row[:1, :P], tot[:1, :1], start=True, stop=True)
                rb = wtmp.tile([P, 1], F32, tag="nrm_rb")
                nc.vector.tensor_copy(rb[:P, :1], pb[:P, :1])
                nc.vector.tensor_scalar(dst_ap, sg[:P, :F], rb[:P, :1], extra_scale,
                                        op0=ALU.mult, op1=ALU.mult)

            wq_n = ps.tile([DL, DIN], F32)
            wk_n = ps.tile([DL, DIN], F32)
            normed(wq_n[:], load_w("q_local_w")[:], DL, DIN, cfg.INV)
            nonneg(wk_n[:], load_w("k_local_w")[:], DL, DIN)

            wego = load_w("w_ego")
            wqg_n = ps.tile([DG, DIN], F32)
            normed(wqg_n[:], load_w("q_global_w")[:], DG, DIN, 1.0)

            wkg_n = ps.tile([DG, DIN], F32)
            nonneg(wkg_n[:], load_w("k_global_w")[:], DG, DIN)

            # transposed weight chunks (all base partition 0)
            wcatT_tab = ps.tile([128, DIN], F32)      # chunk cc: [WqT | WkT]
            wegoT = ps.tile([128, KC * DE], F32)
            wqgT = ps.tile([128, KC * DG], F32)
            wkgT = ps.tile([128, KC * DG], F32)
            for cc in range(KC):
                ch = slice(cc * 128, (cc + 1) * 128)
                pt = psA.tile([128, DL], F32, tag="a")
                nc.tensor.transpose(pt[:, 0:DL], wq_n[:, ch], ident[:DL, :DL])
                nc.scalar.copy(wcatT_tab[:, cc * 128:cc * 128 + DL], pt[:, 0:DL])
                pt = psA.tile([128, DL], F32, tag="a")
                nc.tensor.transpose(pt[:, 0:DL], wk_n[:, ch], ident[:DL, :DL])
                nc.scalar.copy(wcatT_tab[:, cc * 128 + DL:(cc + 1) * 128], pt[:, 0:DL])
                pt = psA.tile([128, DE], F32, tag="a")
                nc.tensor.transpose(pt[:, 0:DE], wego[:, ch], ident[:DE, :DE])
                nc.scalar.copy(wegoT[:, cc * DE:(cc + 1) * DE], pt[:, 0:DE])
                pt = psA.tile([128, DG], F32, tag="a")
                nc.tensor.transpose(pt[:, 0:DG], wqg_n[:, ch], ident[:DG, :DG])
                nc.scalar.copy(wqgT[:, cc * DG:(cc + 1) * DG], pt[:, 0:DG])
                pt = psA.tile([128, DG], F32, tag="a")
                nc.tensor.transpose(pt[:, 0:DG], wkg_n[:, ch], ident[:DG, :DG])
                nc.scalar.copy(wkgT[:, cc * DG:(cc + 1) * DG], pt[:, 0:DG])

            def vT(nm, DD):
                vt = ps.tile([DD, DOUT], F32, tag=f"vt_{nm}")
                wn = wtmp.tile([128, (DOUT // 128) * DD], F32, tag=f"vn_{nm}")
                for a in range(DOUT // 128):
                    wch = wtmp.tile([128, DD], F32, tag="vch")
                    nc.sync.dma_start(wch[:], wts[nm][a * 128:(a + 1) * 128, :])
                    nonneg(wn[:, a * DD:(a + 1) * DD], wch[:], 128, DD)
                    pt = psA.tile([DD, 128], F32, tag="a")
                    nc.tensor.transpose(pt[0:DD, :], wn[:, a * DD:(a + 1) * DD], ident[:])
                    nc.scalar.copy(vt[:, a * 128:(a + 1) * 128], pt[0:DD, :])
                return vt

            VeT = vT("v_ego_w", DE)
            VlT = vT("v_local_w", DL)
            VgT = vT("v_global_w", DG)

            nb = ps.tile([1, DOUT], F32)
            nonneg(nb[:], load_w("bias_b")[:], 1, DOUT)
            bias_bc = ps.tile([128, DOUT], F32)
            pbias = psA.tile([128, DOUT], F32, tag="a")
            nc.tensor.matmul(pbias[:, :], ones_row[:1, :], nb[:1, :], start=True, stop=True)
            nc.scalar.copy(bias_bc[:], pbias[:, :])

            egoT = ps.tile([DE, NS], F32)
            qgT = ps.tile([DG, NS], F32)

            # ---- stage 1: projections per node tile ----
            psum_xbar = psX.tile([1, DIN], F32)
            for i in range(cfg.N_TILES):
                nt = min(128, NS - i * 128)
                xt = s1p.tile([128, DIN], F32, tag="xt")
                nc.sync.dma_start(xt[:nt, :], t_x[i * 128:i * 128 + nt, :])
                nc.tensor.matmul(psum_xbar[:1, :], ones_col[:nt, :1], xt[:nt, :],
                                 start=(i == 0), stop=(i == cfg.N_TILES - 1),
                                 skip_group_check=True)
                xT = s1p.tile([128, KC * 128], F32, tag="xT")
                for cc in range(KC):
                    pt = psA.tile([128, 128], F32, tag="a")
                    nc.tensor.transpose(pt[:, 0:nt], xt[:nt, cc * 128:(cc + 1) * 128], ident[:nt, :nt])
                    nc.scalar.copy(xT[:, cc * 128:cc * 128 + nt], pt[:, 0:nt])
                ptab = psB.tile([128, DQK], F32, tag="b")
                for cc in range(KC):
                    nc.tensor.matmul(ptab[:nt, :], xT[:, cc * 128:cc * 128 + nt],
                                     wcatT_tab[:, cc * 128:(cc + 1) * 128],
                                     start=(cc == 0), stop=(cc == KC - 1))
                tabt = s1p.tile([128, DQK], BF16, tag="tabt")
                nc.scalar.copy(tabt[:nt, :], ptab[:nt, :])
                nc.sync.dma_start(cc_in[i * 128:i * 128 + nt, :], tabt[:nt, :])
                psc_e = psB.tile([DE, 128], F32, tag="b")
                for cc in range(KC):
                    nc.tensor.matmul(psc_e[:, 0:nt], wegoT[:, cc * DE:(cc + 1) * DE],
                                     xT[:, cc * 128:cc * 128 + nt],
                                     start=(cc == 0), stop=(cc == KC - 1))
                nc.scalar.copy(egoT[:, i * 128:i * 128 + nt], psc_e[0:DE, 0:nt])
                psc_g = psB.tile([DG, 128], F32, tag="b")
                for cc in range(KC):
                    nc.tensor.matmul(psc_g[:, 0:nt], wqgT[:, cc * DG:(cc + 1) * DG],
                                     xT[:, cc * 128:cc * 128 + nt],
                                     start=(cc == 0), stop=(cc == KC - 1))
                nc.scalar.copy(qgT[:, i * 128:i * 128 + nt], psc_g[0:DG, 0:nt])

            # ---- stage C: collectives & global branch ----
            xbar_sb = ps.tile([1, DIN], F32)
            nc.scalar.copy(xbar_sb[:], psum_xbar[:1, :])
            nc.sync.dma_start(ar_in[:, :], xbar_sb[:])
            if fake_cc:
                # single-core timing build: stand in for the collectives with
                # equivalent-volume DMA traffic
                nc.sync.dma_start(ar_out[:, :], ar_in[:, :])
                for r in range(cfg.CORES):
                    nc.sync.dma_start(cc_out[r * cfg.NB:(r + 1) * cfg.NB, :], cc_in[:, :])
            else:
                nc.gpsimd.collective_compute("AllReduce", ALU.add, replica_groups=rg,
                                             ins=[ar_in.opt()], outs=[ar_out.opt()])
                nc.gpsimd.collective_compute("AllGather", ALU.bypass, replica_groups=rg,
                                             ins=[cc_in.opt()], outs=[cc_out.opt()])
            xbar_l = ps.tile([1, DIN], F32)
            nc.sync.dma_start(xbar_l[:], ar_out[:, :])
            pkg = psA.tile([DG, 1], F32, tag="a")
            for cc in range(KC):
                pxc = psA.tile([128, 1], F32, tag="a")
                nc.tensor.matmul(pxc[:, :1], xbar_l[0:1, cc * 128:(cc + 1) * 128],
                                 ident[0:1, 0:1], start=True, stop=True)
                xbc = wtmp.tile([128, 1], F32, tag="xbc")
                nc.scalar.copy(xbc[:, :1], pxc[:, :1])
                nc.tensor.matmul(pkg[:DG, :1], wkgT[:, cc * DG:(cc + 1) * DG],
                                 xbc[:, :1], start=(cc == 0), stop=(cc == KC - 1))
            kg = ps.tile([DG, 1], F32)
            nc.vector.tensor_scalar_mul(kg[:], pkg[:DG, :1], cfg.INV / cfg.N)
            nc.scalar.activation(egoT[:], egoT[:], AF.Square, scale=1.0 / DIN)

            # ---- stage G/F: gather, combine, reduce, project ----
            for g in range(cfg.G_TILES):
                idx_sb = {}
                for nm in ("tlo", "thi", "slo", "shi"):
                    it = gp.tile([128, J // 16], I16, tag=f"i_{nm}")
                    nc.sync.dma_start(it[:], t_idx[nm][g])
                    idx_sb[nm] = it
                qlo = gp.tile([128, C * DL], BF16, tag="qlo")
                qhi = gp.tile([128, C * DL], BF16, tag="qhi")
                klo = gp.tile([128, C * DL], BF16, tag="klo")
                khi = gp.tile([128, C * DL], BF16, tag="khi")
                for (dst, idxnm, lohi, col0) in (
                    (qlo, "tlo", 0, 0), (qhi, "thi", 1, 0),
                    (klo, "slo", 0, DL), (khi, "shi", 1, DL),
                ):
                    src = cc_out[cfg.SPLIT:, col0:col0 + DL] if lohi else cc_out[:, col0:col0 + DL]
                    _dma_gather_hbm(nc.gpsimd,
                                    dst[:].rearrange("p (c d) -> p c d", d=DL),
                                    src, idx_sb[idxnm][:], J, J, DL, DQK)
                nc.vector.tensor_add(qlo[:], qlo[:], qhi[:])
                nc.vector.tensor_add(klo[:], klo[:], khi[:])
                ls = qhi
                nc.vector.tensor_mul(ls[:], qlo[:], klo[:])
                lu = gp.tile([128, NPP * DL], F32, tag="lu")
                nc.vector.tensor_reduce(
                    lu[:].rearrange("p (g2 d) -> p g2 d", g2=NPP),
                    ls[:].rearrange("p (g2 j d) -> p g2 d j", g2=NPP, j=DEG, d=DL),
                    mybir.AxisListType.X, ALU.add)
                tgs = []
                for g2 in range(NPP):
                    ptg = psB.tile([DL, 128], F32, tag="b")
                    nc.tensor.transpose(ptg[0:DL, :], lu[:, g2 * DL:(g2 + 1) * DL], ident[:])
                    tg = fp.tile([DL, 128], F32, tag=f"tg{g2}")
                    nc.scalar.copy(tg[:, :], ptg[0:DL, :])
                    tgs.append(tg)

                for g2 in range(NPP):
                    t = g * NPP + g2
                    if t >= cfg.N_TILES:
                        break
                    nt = min(128, NS - t * 128)
                    colsl = slice(t * 128, t * 128 + nt)
                    pss = psS.tile([128, 1], F32, tag="pss")
                    pout = psO.tile([128, DOUT], F32, tag="pout")
                    eT = egoT[:, colsl]
                    lT = tgs[g2][:, 0:nt]
                    gtmp = fp.tile([DG, 128], F32, tag="gtmp")
                    nc.vector.tensor_scalar_mul(gtmp[:, 0:nt], qgT[:, colsl], kg[:, 0:1])
                    gT = gtmp[:, 0:nt]
                    nc.tensor.matmul(pss[:nt, :1], eT, ones_col[:DE, :1], start=True, stop=False)
                    nc.tensor.matmul(pout[:nt, :], eT, VeT[:, :], start=True, stop=False)
                    nc.tensor.matmul(pss[:nt, :1], lT, ones_col[:DL, :1], start=False, stop=False)
                    nc.tensor.matmul(pout[:nt, :], lT, VlT[:, :], start=False, stop=False)
                    nc.tensor.matmul(pss[:nt, :1], gT, ones_col[:DG, :1], start=False, stop=True)
                    nc.tensor.matmul(pout[:nt, :], gT, VgT[:, :], start=False, stop=True)
                    rr = fp.tile([128, 1], F32, tag="rr")
                    nc.vector.tensor_scalar_add(rr[:nt, :1], pss[:nt, :1], 0.001)
                    nc.vector.reciprocal(rr[:nt, :1], rr[:nt, :1])
                    osb = fp.tile([128, DOUT], F32, tag="osb")
                    nc.vector.tensor_scalar_mul(osb[:nt, :], pout[:nt, :], rr[:nt, 0:1])
                    nc.vector.tensor_add(osb[:nt, :], osb[:nt, :], bias_bc[:nt, :])
                    nc.sync.dma_start(t_res[t * 128:t * 128 + nt, :], osb[:nt, :])

    nc.compile()
    return nc


# ----------------------------------------------------------------------------
# entry point
# ----------------------------------------------------------------------------
_CACHE = {}


def _get_built(cfg_key=None):
    if "nc" not in _CACHE:
        cfg = Cfg()
        _CACHE["cfg"] = cfg
        _CACHE["nc"] = build(cfg)
    return _CACHE["cfg"], _CACHE["nc"]


def kernel(adj_matrix, x, w_ego, v_ego_w, q_local_w, k_local_w, v_local_w,
           q_global_w, k_global_w, v_global_w, bias_b):
    cfg, nc = _get_built()
    adj = np.asarray(adj_matrix)
    x = np.asarray(x, dtype=np.float32)
    weights = {
        "w_ego": np.asarray(w_ego, np.float32),
        "v_ego_w": np.asarray(v_ego_w, np.float32),
        "q_local_w": np.asarray(q_local_w, np.float32),
        "k_local_w": np.asarray(k_local_w, np.float32),
        "v_local_w": np.asarray(v_local_w, np.float32),
        "q_global_w": np.asarray(q_global_w, np.float32),
        "k_global_w": np.asarray(k_global_w, np.float32),
        "v_global_w": np.asarray(v_global_w, np.float32),
        "bias_b": np.asarray(bias_b, np.float32),
    }
    in_maps = []
    for c in range(cfg.CORES):
        m = prep_core_inputs(cfg, adj, x, c)
        m.update(weights)
        in_maps.append(m)
    res = run_bass_kernel_spmd(nc, in_maps, core_ids=list(range(cfg.CORES)))
    return np.concatenate([res.results[c]["res"] for c in range(cfg.CORES)], axis=0)
